# revision 1
# baseline (speedup 1.0000x reference)
"""DCM (dynamic conv module) Trainium2 kernel.

Reference computation (per sample b, channel c):
  f[b,c,3,3]  = adaptive_avg_pool2d(x[b,c], 3)        # dynamic depthwise filter
  out[b,c]    = depthwise_conv3x3(x[b,c], f[b,c])     # zero padding 1
  y           = relu(batchnorm_train(out, gamma, beta))  # batch stats over (B,H,W)

Sharding: data-parallel over batch B=16 across 8 cores (2 samples/core).
Sync-BN via a [C,2] AllReduce of per-channel (sum, sumsq).

Per-core layout: channels C=128 on partitions, free dim = H*W per sample.
Conv = 9 shifted taps done as diag(f_tap) matmuls (float32r, 1 cyc/row)
accumulated in PSUM; horizontal zero-padding handled by letting taps wrap
across row ends and subtracting the wrapped term on the two edge columns
(DVE scalar_tensor_tensor fixups). Conv output stays resident in SBUF
(128 KiB/partition) until the stats AllReduce, then BN+ReLU is applied
in-place (ACT/DVE split) and DMA'd out.
"""

import os
import numpy as np

# ---------------------------------------------------------------- constants
B, C, H, W = 16, 128, 128, 128
N_CORES = 8
BL = B // N_CORES          # samples per core
HW = H * W                 # 16384 free elems per plane
FS = 3
BN_EPS = 1e-5

ROWS = 16                  # output rows per psum tile
NCHUNK = H // ROWS         # 8 chunks per plane
TILE_F = ROWS * W          # 2048 free elems per psum tile
XT_F = (ROWS + 2) * W + 2  # x chunk with halo rows + 1 elem pad each end
NPSUM = NCHUNK * BL        # psum tiles per core

# adaptive_avg_pool2d(3) bin boundaries (PyTorch convention)
SH = [(i * H) // FS for i in range(FS)]
EH = [-((-(i + 1) * H) // FS) for i in range(FS)]
SW = [(i * W) // FS for i in range(FS)]
EW = [-((-(i + 1) * W) // FS) for i in range(FS)]

TAPS = [(di, dj) for di in (-1, 0, 1) for dj in (-1, 0, 1)]  # t = 3*(di+1)+(dj+1)

MM_N = 512                 # fp32 moving-operand max per matmul
NSL = TILE_F // MM_N       # bank slices per psum tile

# Matmul operand dtype for the conv taps. The PE runs fp32 at 4 cycles/row;
# float32r (rounded fp32) and bf16 run at 1 cycle/row. float32r operands must
# be produced as float32r per the BIR verifier, so the tap pass streams x a
# second time from a dedicated DRAM tensor declared at this dtype.
MM_DTYPE = os.environ.get("DCM_MM_DTYPE", "f32r")


def _counts_recip():
    cr = np.empty((C, FS * FS), dtype=np.float32)
    for i in range(FS):
        for j in range(FS):
            cr[:, 3 * i + j] = 1.0 / float((EH[i] - SH[i]) * (EW[j] - SW[j]))
    return cr


def build_nc(n_cores: int = N_CORES):
    """Build + compile the per-core Bass program (identical on all cores)."""
    import concourse.bacc as bacc
    import concourse.tile as tile
    from concourse import mybir

    f32 = mybir.dt.float32
    f32r = mybir.dt.float32r
    AT = mybir.ActivationFunctionType
    OP = mybir.AluOpType
    AX = mybir.AxisListType

    ntot = float(n_cores * BL * HW)   # BN element count per channel

    nc = bacc.Bacc(
        "TRN2",
        target_bir_lowering=False,
        debug=False,
        num_devices=n_cores,
    )

    if MM_DTYPE == "f32r":
        mdt = mybir.dt.float32r
    elif MM_DTYPE == "bf16":
        mdt = mybir.dt.bfloat16
    else:
        mdt = f32

    x_d = nc.dram_tensor("x", [BL, C, HW], f32, kind="ExternalInput").ap()
    x2_d = (
        nc.dram_tensor("x2", [BL, C, HW], mdt, kind="ExternalInput").ap()
        if MM_DTYPE != "f32"
        else x_d
    )
    gamma_d = nc.dram_tensor("gamma", [C, 1], f32, kind="ExternalInput").ap()
    beta_d = nc.dram_tensor("beta", [C, 1], f32, kind="ExternalInput").ap()
    ident_d = nc.dram_tensor("ident", [C, C], f32, kind="ExternalInput").ap()
    crecip_d = nc.dram_tensor("crecip", [C, FS * FS], f32, kind="ExternalInput").ap()
    y_d = nc.dram_tensor("y", [BL, C, HW], f32, kind="ExternalOutput").ap()

    with tile.TileContext(nc) as tc:
        with (
            tc.tile_pool(name="singles", bufs=1) as singles,
            tc.tile_pool(name="xpool", bufs=2) as xpool,
            tc.tile_pool(name="outres", bufs=NPSUM) as outres,
            tc.tile_pool(name="psum", bufs=2, space="PSUM") as psum,
            tc.tile_pool(name="colsp", bufs=2) as colsp,
            tc.tile_pool(name="fpool", bufs=2) as fpool,
            tc.tile_pool(name="diagp", bufs=2 * FS * FS) as diagp,
            tc.tile_pool(name="statp", bufs=1) as statp,
            tc.tile_pool(name="dram", bufs=1, space="DRAM") as dram,
        ):
            # ---- constants
            gamma_s = singles.tile([C, 1], f32, tag="gamma")
            nc.sync.dma_start(out=gamma_s[:], in_=gamma_d[:, :])
            beta_s = singles.tile([C, 1], f32, tag="beta")
            nc.sync.dma_start(out=beta_s[:], in_=beta_d[:, :])
            ident_s = singles.tile([C, C], f32, tag="ident")
            nc.sync.dma_start(out=ident_s[:], in_=ident_d[:, :])
            crecip_s = singles.tile([C, FS * FS], f32, tag="crecip")
            nc.sync.dma_start(out=crecip_s[:], in_=crecip_d[:, :])

            sums = statp.tile([C, NPSUM], f32, tag="sums")
            sumsq = statp.tile([C, NPSUM], f32, tag="sumsq")

            # Dummy warm-up AllReduce issued at kernel start: absorbs the
            # one-time ncfw ramp so the real stats AllReduce on the critical
            # path is cheaper. Runs concurrently with the pooling pass.
            warm = statp.tile([C, 2], f32, tag="warm")
            nc.gpsimd.memset(warm[:], 0.0)
            dw_in = dram.tile([C, 2], f32, tag="dw_in")
            dw_out = dram.tile([C, 2], f32, tag="dw_out")
            nc.sync.dma_start(out=dw_in[:], in_=warm[:])
            nc.gpsimd.collective_compute(
                "AllReduce",
                OP.add,
                replica_groups=[list(range(n_cores))],
                ins=[dw_in[:].opt()],
                outs=[dw_out[:].opt()],
            )

            out_tiles = []
            kpt = 0  # global psum-tile index

            def load_chunk(s, c, src=x_d, dtype=f32, tag="xt"):
                """DMA one halo chunk of plane s into a fresh x tile."""
                xt = xpool.tile([C, XT_F], dtype, tag=tag)
                # float32r has no memset encoding; same bits as f32
                mview = xt[:].bitcast(f32) if dtype == mybir.dt.float32r else xt[:]
                r_lo = c * ROWS - 1
                r_hi = c * ROWS + ROWS + 1
                # 1-elem pads at both ends (read by corner-wrap taps; must be
                # finite so the fixup subtraction cancels exactly). On DVE —
                # gpsimd can be blocked for long stretches by the collective.
                nc.vector.memset(mview[:, 0:1], 0.0)
                nc.vector.memset(mview[:, XT_F - 1:XT_F], 0.0)
                if r_lo < 0:
                    nc.vector.memset(mview[:, 1:1 + W], 0.0)
                if r_hi > H:
                    nc.vector.memset(mview[:, 1 + (ROWS + 1) * W:1 + (ROWS + 2) * W], 0.0)
                src_lo = max(r_lo, 0) * W
                src_hi = min(r_hi, H) * W
                dst_lo = 1 + (max(r_lo, 0) - r_lo) * W
                nc.sync.dma_start(
                    out=xt[:, dst_lo:dst_lo + (src_hi - src_lo)],
                    in_=src[s, :, src_lo:src_hi],
                )
                return xt

            for s in range(BL):
                # ---------------- phase 1: pooling pass over plane s
                # (no halo needed; plain 16-row tiles, triple buffered so the
                # DMA cadence, not the pool slots, paces the pipeline)
                colS = colsp.tile([C, FS, H], f32, tag="colS")
                for c in range(NCHUNK):
                    xt = xpool.tile([C, TILE_F], f32, tag="xt", bufs=3)
                    nc.sync.dma_start(
                        out=xt[:], in_=x_d[s, :, c * TILE_F:(c + 1) * TILE_F]
                    )
                    xv = xt[:].rearrange("p (r w) -> p r w", w=W)
                    for j in range(FS):
                        nc.vector.tensor_reduce(
                            out=colS[:, j, c * ROWS:(c + 1) * ROWS],
                            in_=xv[:, :, SW[j]:EW[j]],
                            axis=AX.X,
                            op=OP.add,
                        )

                # ---------------- filter f [C,9] and diag weights
                fT = fpool.tile([C, FS * FS], f32, tag="fT")
                for i in range(FS):
                    for j in range(FS):
                        k = 3 * i + j
                        nc.vector.tensor_reduce(
                            out=fT[:, k:k + 1],
                            in_=colS[:, j, SH[i]:EH[i]],
                            axis=AX.X,
                            op=OP.add,
                        )
                nc.vector.tensor_mul(fT[:], fT[:], crecip_s[:])
                # fixup scalars at the matmul operand precision so the
                # subtraction matches what the PE added
                fneg = fpool.tile(
                    [C, FS * FS], mdt if MM_DTYPE == "bf16" else f32, tag="fneg"
                )
                nc.vector.tensor_scalar_mul(fneg[:], fT[:], -1.0)
                diags = []
                for t in range(FS * FS):
                    dg = diagp.tile([C, C], mdt, tag="diag")
                    nc.vector.tensor_scalar_mul(dg[:], ident_s[:], fT[:, t:t + 1])
                    diags.append(dg)

                # ---------------- phase 2: conv taps (x streamed a 2nd time)
                for c in range(NCHUNK):
                    xt = load_chunk(s, c, src=x2_d, dtype=mdt, tag="xt2")
                    # DVE fixups read the same tile; f32r is bit-identical f32
                    xtv = xt[:].bitcast(f32) if MM_DTYPE == "f32r" else xt[:]
                    pt = psum.tile([C, TILE_F], f32, tag="pt")
                    for sl in range(NSL):
                        for t, (di, dj) in enumerate(TAPS):
                            base = 1 + (di + 1) * W + dj + sl * MM_N
                            nc.tensor.matmul(
                                pt[:, sl * MM_N:(sl + 1) * MM_N],
                                diags[t][:],
                                xt[:, base:base + MM_N],
                                start=(t == 0),
                                stop=(t == FS * FS - 1),
                            )
                    # edge-column fixups: subtract the horizontally wrapped term
                    pv = pt[:].rearrange("p (r w) -> p r w", w=W)
                    for i, di in enumerate((-1, 0, 1)):
                        # w = 0 read x[h+di, -1] -> wrapped to (h+di-1, W-1)
                        src = xtv[:, (di + 1) * W:(di + 1) * W + ROWS * W].rearrange(
                            "p (r w) -> p r w", w=W
                        )[:, :, 0:1]
                        nc.vector.scalar_tensor_tensor(
                            out=pv[:, :, 0:1],
                            in0=src,
                            scalar=fneg[:, 3 * i:3 * i + 1],
                            in1=pv[:, :, 0:1],
                            op0=OP.mult,
                            op1=OP.add,
                        )
                        # w = W-1 read x[h+di, W] -> wrapped to (h+di+1, 0),
                        # i.e. flat cells 1+(di+2)*W + r*W; expressed as col
                        # W-1 of a view starting 127 elems earlier
                        s0 = (di + 1) * W + 2
                        src = xtv[:, s0:s0 + ROWS * W].rearrange(
                            "p (r w) -> p r w", w=W
                        )[:, :, W - 1:W]
                        nc.vector.scalar_tensor_tensor(
                            out=pv[:, :, W - 1:W],
                            in0=src,
                            scalar=fneg[:, 3 * i + 2:3 * i + 3],
                            in1=pv[:, :, W - 1:W],
                            op0=OP.mult,
                            op1=OP.add,
                        )
                    # PSUM -> resident SBUF copy, fused per-channel sum
                    ot = outres.tile([C, TILE_F], f32, tag="ot")
                    nc.scalar.activation(
                        out=ot[:], in_=pt[:], func=AT.Copy,
                        accum_out=sums[:, kpt:kpt + 1],
                    )
                    # sum of squares; squared values overwrite the psum tile
                    # in place (only accum_out is kept)
                    nc.scalar.activation(
                        out=pt[:], in_=pt[:], func=AT.Square,
                        accum_out=sumsq[:, kpt:kpt + 1],
                    )
                    out_tiles.append((s, c, ot))
                    kpt += 1

            # ---------------- sync-BN stats AllReduce
            arin = statp.tile([C, 2], f32, tag="arin")
            nc.vector.tensor_reduce(out=arin[:, 0:1], in_=sums[:], axis=AX.X, op=OP.add)
            nc.vector.tensor_reduce(out=arin[:, 1:2], in_=sumsq[:], axis=AX.X, op=OP.add)
            d_in = dram.tile([C, 2], f32, tag="d_in")
            d_out = dram.tile([C, 2], f32, tag="d_out")
            nc.sync.dma_start(out=d_in[:], in_=arin[:])
            nc.gpsimd.collective_compute(
                "AllReduce",
                OP.add,
                replica_groups=[list(range(n_cores))],
                ins=[d_in[:].opt()],
                outs=[d_out[:].opt()],
            )
            aro = statp.tile([C, 2], f32, tag="aro")
            nc.sync.dma_start(out=aro[:], in_=d_out[:])

            # ---------------- BN scale/shift (all [C,1], fp32)
            mean = statp.tile([C, 1], f32, tag="mean")
            nc.vector.tensor_scalar_mul(mean[:], aro[:, 0:1], 1.0 / ntot)
            ex2 = statp.tile([C, 1], f32, tag="ex2")
            nc.vector.tensor_scalar_mul(ex2[:], aro[:, 1:2], 1.0 / ntot)
            var = statp.tile([C, 1], f32, tag="var")
            nc.vector.tensor_mul(var[:], mean[:], mean[:])
            nc.vector.tensor_sub(var[:], ex2[:], var[:])
            veps = statp.tile([C, 1], f32, tag="veps")
            nc.vector.tensor_scalar_add(veps[:], var[:], BN_EPS)
            eps_t = statp.tile([C, 1], f32, tag="eps_t")
            nc.vector.memset(eps_t[:], BN_EPS)
            sd = statp.tile([C, 1], f32, tag="sd")
            nc.scalar.activation(out=sd[:], in_=var[:], func=AT.Sqrt, bias=eps_t[:])
            z = statp.tile([C, 1], f32, tag="z")
            nc.vector.reciprocal(z[:], sd[:])
            # one Newton step: z <- z * (1.5 - 0.5 * veps * z^2)
            nt = statp.tile([C, 1], f32, tag="nt")
            nc.vector.tensor_mul(nt[:], z[:], z[:])
            nc.vector.tensor_mul(nt[:], nt[:], veps[:])
            nc.vector.tensor_scalar(
                out=nt[:], in0=nt[:], scalar1=-0.5, scalar2=1.5,
                op0=OP.mult, op1=OP.add,
            )
            nc.vector.tensor_mul(z[:], z[:], nt[:])
            scale_t = statp.tile([C, 1], f32, tag="scale_t")
            nc.vector.tensor_mul(scale_t[:], gamma_s[:], z[:])
            shift_t = statp.tile([C, 1], f32, tag="shift_t")
            nc.vector.tensor_mul(shift_t[:], mean[:], scale_t[:])
            nc.vector.tensor_sub(shift_t[:], beta_s[:], shift_t[:])

            # ---------------- BN apply + ReLU + writeback (ACT / DVE split;
            # DVE needs 2 ops per tile vs ACT's 1, so ACT takes ~10/16)
            for idx, (s, c, ot) in enumerate(out_tiles):
                if idx % 8 < 5:
                    nc.scalar.activation(
                        out=ot[:], in_=ot[:], func=AT.Relu,
                        scale=scale_t[:], bias=shift_t[:],
                    )
                else:
                    nc.vector.tensor_scalar(
                        out=ot[:], in0=ot[:],
                        scalar1=scale_t[:], scalar2=shift_t[:],
                        op0=OP.mult, op1=OP.add,
                    )
                    nc.vector.tensor_scalar_max(ot[:], ot[:], 0.0)
                nc.sync.dma_start(
                    out=y_d[s, :, c * TILE_F:(c + 1) * TILE_F], in_=ot[:],
                )

    nc.compile()
    return nc


_NC_CACHE = {}


def _get_nc(n_cores: int = N_CORES):
    if n_cores not in _NC_CACHE:
        _NC_CACHE[n_cores] = build_nc(n_cores)
    return _NC_CACHE[n_cores]


def make_in_maps(x: np.ndarray, gamma: np.ndarray, beta: np.ndarray,
                 n_cores: int = N_CORES):
    x_r = np.ascontiguousarray(
        np.asarray(x, dtype=np.float32).reshape(B, C, HW)
    )
    g = np.ascontiguousarray(np.asarray(gamma, dtype=np.float32).reshape(C, 1))
    b = np.ascontiguousarray(np.asarray(beta, dtype=np.float32).reshape(C, 1))
    ident = np.eye(C, dtype=np.float32)
    crecip = _counts_recip()
    if MM_DTYPE == "bf16":
        import ml_dtypes

        x2 = x_r.astype(ml_dtypes.bfloat16)
    elif MM_DTYPE == "f32r":
        x2 = x_r  # float32r is bit-identical to float32
    else:
        x2 = None
    maps = []
    for core in range(n_cores):
        m = {
            "x": x_r[core * BL:(core + 1) * BL],
            "gamma": g,
            "beta": b,
            "ident": ident,
            "crecip": crecip,
        }
        if x2 is not None:
            m["x2"] = x2[core * BL:(core + 1) * BL]
        maps.append(m)
    return maps


def kernel(x, gamma, beta):
    from concourse import bass_utils

    nc = _get_nc(N_CORES)
    in_maps = make_in_maps(x, gamma, beta, N_CORES)
    res = bass_utils.run_bass_kernel_spmd(nc, in_maps, core_ids=list(range(N_CORES)))
    y = np.concatenate([res.results[c]["y"] for c in range(N_CORES)], axis=0)
    return y.reshape(B, C, H, W).astype(np.float32)



# revision 3
# speedup vs baseline: 1.2166x; 1.2166x over previous
"""DCM (dynamic conv module) Trainium2 kernel, bf16 multi-engine version.

Reference computation (per sample b, channel c):
  f[b,c,3,3]  = adaptive_avg_pool2d(x[b,c], 3)        # dynamic depthwise filter
  out[b,c]    = depthwise_conv3x3(x[b,c], f[b,c])     # zero padding 1
  y           = relu(batchnorm_train(out, gamma, beta))  # batch stats over (B,H,W)

Sharding: data-parallel over batch B=16 across 8 cores (2 samples/core).
Sync-BN via a [C,2] AllReduce of per-channel (sum, sumsq).

Per-core dataflow (all bf16 except PSUM/stats; tolerance is 2e-2):
  - x plane [C, H*W] bf16 resident in SBUF (single HBM read).
  - pooling: DVE tensor_reduce in bf16 (2x mode) -> colS -> f.
  - conv: 9 taps split across engines.
      PE (6 taps): diag(f_t) matmuls, 512-col slices, accumulated in PSUM.
        Zero padding by row-range clipping; horizontal wrap of dj=+-1 taps
        fixed by tiny DVE column fixups on the resident output.
      DVE (3 corner taps): scalar_tensor_tensor into a bf16 SBUF accumulator
        (2x mode); clipped views make padding exact.
      PE identity matmul folds the DVE accumulator into PSUM.
  - ACT: PSUM->SBUF bf16 copy with accum_out=sum, square with accum_out=sumsq.
  - sync-BN AllReduce of [C,2] (warmup AR at start absorbs ncfw ramp).
  - ACT: relu(scale*out+shift) -> bf16 y tiles -> HBM.
"""

import numpy as np

# ---------------------------------------------------------------- constants
B, C, H, W = 16, 128, 128, 128
N_CORES = 8
BL = B // N_CORES          # samples per core
HW = H * W                 # 16384 free elems per plane
FS = 3
BN_EPS = 1e-5

ROWS = 16                  # output rows per psum tile
NCHUNK = H // ROWS         # 8 chunks per plane
TILE_F = ROWS * W          # 2048 free elems per psum tile
NPSUM = NCHUNK * BL        # psum tiles per core

# adaptive_avg_pool2d(3) bin boundaries (PyTorch convention)
SH = [(i * H) // FS for i in range(FS)]
EH = [-((-(i + 1) * H) // FS) for i in range(FS)]
SW = [(i * W) // FS for i in range(FS)]
EW = [-((-(i + 1) * W) // FS) for i in range(FS)]

# tap index t = 3*(di+1)+(dj+1)
def tidx(di, dj):
    return 3 * (di + 1) + (dj + 1)

# engine split: PE does the cross taps + one corner, DVE the other corners
PE_TAPS = [(0, 0), (-1, 0), (1, 0), (0, -1), (0, 1), (1, 1)]
DVE_TAPS = [(-1, -1), (1, -1), (-1, 1)]

MM_N = 512                 # columns per matmul (one psum bank)
NSL = TILE_F // MM_N       # bank slices per psum tile


def _counts_recip():
    cr = np.empty((C, FS * FS), dtype=np.float32)
    for i in range(FS):
        for j in range(FS):
            cr[:, 3 * i + j] = 1.0 / float((EH[i] - SH[i]) * (EW[j] - SW[j]))
    return cr


def build_nc(n_cores: int = N_CORES):
    """Build + compile the per-core Bass program (identical on all cores)."""
    import concourse.bacc as bacc
    import concourse.tile as tile
    from concourse import mybir

    f32 = mybir.dt.float32
    bf16 = mybir.dt.bfloat16
    AT = mybir.ActivationFunctionType
    OP = mybir.AluOpType
    AX = mybir.AxisListType

    ntot = float(n_cores * BL * HW)   # BN element count per channel

    nc = bacc.Bacc(
        "TRN2",
        target_bir_lowering=False,
        debug=False,
        num_devices=n_cores,
    )

    x_d = nc.dram_tensor("x", [BL, C, HW], bf16, kind="ExternalInput").ap()
    gamma_d = nc.dram_tensor("gamma", [C, 1], f32, kind="ExternalInput").ap()
    beta_d = nc.dram_tensor("beta", [C, 1], f32, kind="ExternalInput").ap()
    ident_d = nc.dram_tensor("ident", [C, C], bf16, kind="ExternalInput").ap()
    crecip_d = nc.dram_tensor("crecip", [C, FS * FS], f32, kind="ExternalInput").ap()
    y_d = nc.dram_tensor("y", [BL, C, HW], bf16, kind="ExternalOutput").ap()

    with tile.TileContext(nc) as tc, nc.allow_low_precision("bf16 by design"):
        with (
            tc.tile_pool(name="singles", bufs=1) as singles,
            tc.tile_pool(name="planes", bufs=1) as planes,
            tc.tile_pool(name="outres", bufs=1) as outres,
            tc.tile_pool(name="psum", bufs=2, space="PSUM") as psum,
            tc.tile_pool(name="accp", bufs=3) as accp,
            tc.tile_pool(name="ypool", bufs=4) as ypool,
            tc.tile_pool(name="colsp", bufs=BL) as colsp,
            tc.tile_pool(name="fpool", bufs=BL) as fpool,
            tc.tile_pool(name="diagp", bufs=2 * len(PE_TAPS)) as diagp,
            tc.tile_pool(name="statp", bufs=1) as statp,
            tc.tile_pool(name="dram", bufs=1, space="DRAM") as dram,
        ):
            # ---- constants
            gamma_s = singles.tile([C, 1], f32, tag="gamma")
            nc.sync.dma_start(out=gamma_s[:], in_=gamma_d[:, :])
            beta_s = singles.tile([C, 1], f32, tag="beta")
            nc.sync.dma_start(out=beta_s[:], in_=beta_d[:, :])
            ident_s = singles.tile([C, C], bf16, tag="ident")
            nc.sync.dma_start(out=ident_s[:], in_=ident_d[:, :])
            crecip_s = singles.tile([C, FS * FS], f32, tag="crecip")
            nc.sync.dma_start(out=crecip_s[:], in_=crecip_d[:, :])

            sums = statp.tile([C, NPSUM], f32, tag="sums")
            sumsq = statp.tile([C, NPSUM], f32, tag="sumsq")

            # Dummy warm-up AllReduce issued at kernel start: absorbs the
            # one-time ncfw ramp so the real stats AllReduce on the critical
            # path is cheaper. Runs concurrently with the pooling pass.
            warm = statp.tile([C, 2], f32, tag="warm")
            nc.gpsimd.memset(warm[:], 0.0)
            dw_in = dram.tile([C, 2], f32, tag="dw_in")
            dw_out = dram.tile([C, 2], f32, tag="dw_out")
            nc.sync.dma_start(out=dw_in[:], in_=warm[:])
            nc.gpsimd.collective_compute(
                "AllReduce",
                OP.add,
                replica_groups=[list(range(n_cores))],
                ins=[dw_in[:].opt()],
                outs=[dw_out[:].opt()],
            )

            # ---------------- load planes + pooling + filters
            xs_t = []     # resident bf16 planes
            fT_t = []     # [C,9] f32 filters
            fneg_t = []   # [C,9] f32 negated filters
            dgs_t = []    # per-sample list of bf16 diag weights for PE taps
            for s in range(BL):
                xs = planes.tile([C, HW], bf16, tag=f"xs{s}")
                colS = colsp.tile([C, FS, H], bf16, tag="colS")
                for c in range(NCHUNK):
                    nc.sync.dma_start(
                        out=xs[:, c * TILE_F:(c + 1) * TILE_F],
                        in_=x_d[s, :, c * TILE_F:(c + 1) * TILE_F],
                    )
                    xv = xs[:, c * TILE_F:(c + 1) * TILE_F].rearrange(
                        "p (r w) -> p r w", w=W
                    )
                    for j in range(FS):
                        nc.vector.tensor_reduce(
                            out=colS[:, j, c * ROWS:(c + 1) * ROWS],
                            in_=xv[:, :, SW[j]:EW[j]],
                            axis=AX.X,
                            op=OP.add,
                        )
                fT = fpool.tile([C, FS * FS], f32, tag="fT")
                for i in range(FS):
                    for j in range(FS):
                        k = 3 * i + j
                        nc.vector.tensor_reduce(
                            out=fT[:, k:k + 1],
                            in_=colS[:, j, SH[i]:EH[i]],
                            axis=AX.X,
                            op=OP.add,
                        )
                nc.vector.tensor_mul(fT[:], fT[:], crecip_s[:])
                fneg = fpool.tile([C, FS * FS], f32, tag="fneg")
                nc.vector.tensor_scalar_mul(fneg[:], fT[:], -1.0)
                dgs = []
                for (di, dj) in PE_TAPS:
                    t = tidx(di, dj)
                    dg = diagp.tile([C, C], bf16, tag="diag")
                    # dg = diag(f_t) in bf16 (ACT: ident * per-partition scale)
                    nc.scalar.activation(
                        out=dg[:], in_=ident_s[:], func=AT.Copy,
                        scale=fT[:, t:t + 1],
                    )
                    dgs.append(dg)
                xs_t.append(xs)
                fT_t.append(fT)
                fneg_t.append(fneg)
                dgs_t.append(dgs)

            # ---------------- conv chunks
            out_tiles = []
            for s in range(BL):
                xs, fT, fneg, dgs = xs_t[s], fT_t[s], fneg_t[s], dgs_t[s]
                xr = xs[:].rearrange("p (r w) -> p r w", w=W)   # [C,H,W] view
                ot_full = outres.tile([C, HW], bf16, tag=f"ot{s}")
                for c in range(NCHUNK):
                    R0 = c * ROWS
                    kpt = s * NCHUNK + c

                    # ---- DVE corner taps into bf16 accumulator
                    acc = accp.tile([C, TILE_F], bf16, tag="acc")
                    av = acc[:].rearrange("p (r w) -> p r w", w=W)
                    first = (1, -1) if c == 0 else (-1, -1)
                    di, dj = first
                    t = tidx(di, dj)
                    off = (R0 + di) * W + dj
                    nc.vector.tensor_scalar_mul(
                        acc[:], xs[:, off:off + TILE_F], fT[:, t:t + 1]
                    )
                    # w=0 column of the full view is the wrapped element
                    # x[R+di-? , W-1]; true contribution is the zero pad, so
                    # subtract it back out.
                    xw = xs[:, off:off + TILE_F].rearrange("p (r w) -> p r w", w=W)
                    nc.vector.scalar_tensor_tensor(
                        out=av[:, :, 0:1],
                        in0=xw[:, :, 0:1],
                        scalar=fneg[:, t:t + 1],
                        in1=av[:, :, 0:1],
                        op0=OP.mult,
                        op1=OP.add,
                    )
                    for (di, dj) in DVE_TAPS:
                        if (di, dj) == first:
                            continue
                        t = tidx(di, dj)
                        rlo = max(R0, -di)
                        rhi = min(R0 + ROWS, H - di)
                        wlo = max(0, -dj)
                        whi = W - max(0, dj)
                        nc.vector.scalar_tensor_tensor(
                            out=av[:, rlo - R0:rhi - R0, wlo:whi],
                            in0=xr[:, rlo + di:rhi + di, wlo + dj:whi + dj],
                            scalar=fT[:, t:t + 1],
                            in1=av[:, rlo - R0:rhi - R0, wlo:whi],
                            op0=OP.mult,
                            op1=OP.add,
                        )

                    # ---- PE taps (512-col diag matmuls) + identity fold
                    pt = psum.tile([C, TILE_F], f32, tag="pt")
                    for ti, (di, dj) in enumerate(PE_TAPS):
                        for sl in range(NSL):
                            r0 = R0 + sl * 4
                            rlo = max(r0, -di)
                            rhi = min(r0 + 4, H - di)
                            if rlo >= rhi:
                                continue
                            oc0 = (rlo - R0) * W
                            oc1 = (rhi - R0) * W
                            m0 = (rlo + di) * W + dj
                            m1 = (rhi + di) * W + dj
                            if m0 < 0:
                                oc0 += 1
                                m0 = 0
                            if m1 > HW:
                                oc1 -= 1
                                m1 = HW
                            nc.tensor.matmul(
                                pt[:, oc0:oc1],
                                dgs[ti][:],
                                xs[:, m0:m1],
                                start=(ti == 0),
                                stop=False,
                            )
                    for sl in range(NSL):
                        nc.tensor.matmul(
                            pt[:, sl * MM_N:(sl + 1) * MM_N],
                            ident_s[:],
                            acc[:, sl * MM_N:(sl + 1) * MM_N],
                            start=False,
                            stop=True,
                        )

                    # ---- PSUM -> resident SBUF bf16 copy, fused stats
                    ot = ot_full[:, c * TILE_F:(c + 1) * TILE_F]
                    nc.scalar.activation(
                        out=ot, in_=pt[:], func=AT.Copy,
                        accum_out=sums[:, kpt:kpt + 1],
                    )
                    nc.scalar.activation(
                        out=pt[:], in_=pt[:], func=AT.Square,
                        accum_out=sumsq[:, kpt:kpt + 1],
                    )

                    # ---- horizontal-wrap fixups for PE dj=+-1 taps, applied
                    # to the resident bf16 copy (stats keep the tiny wrap
                    # error: ~1e-7 relative on mean/var, negligible).
                    otv = ot.rearrange("p (r w) -> p r w", w=W)
                    for (di, dj) in PE_TAPS:
                        if dj == 0:
                            continue
                        t = tidx(di, dj)
                        if dj == -1:
                            Rlo = max(R0, 1 - di)
                            Rhi = min(R0 + ROWS, H - di)
                            if Rlo >= Rhi:
                                continue
                            src = xr[:, Rlo + di - 1:Rhi + di - 1, W - 1:W]
                            dst = otv[:, Rlo - R0:Rhi - R0, 0:1]
                        else:
                            Rlo = max(R0, -di)
                            Rhi = min(R0 + ROWS, H - 1 - di)
                            if Rlo >= Rhi:
                                continue
                            src = xr[:, Rlo + di + 1:Rhi + di + 1, 0:1]
                            dst = otv[:, Rlo - R0:Rhi - R0, W - 1:W]
                        nc.vector.scalar_tensor_tensor(
                            out=dst,
                            in0=src,
                            scalar=fneg[:, t:t + 1],
                            in1=dst,
                            op0=OP.mult,
                            op1=OP.add,
                        )
                    out_tiles.append((s, c, ot))

            # ---------------- sync-BN stats AllReduce
            arin = statp.tile([C, 2], f32, tag="arin")
            nc.vector.tensor_reduce(out=arin[:, 0:1], in_=sums[:], axis=AX.X, op=OP.add)
            nc.vector.tensor_reduce(out=arin[:, 1:2], in_=sumsq[:], axis=AX.X, op=OP.add)
            d_in = dram.tile([C, 2], f32, tag="d_in")
            d_out = dram.tile([C, 2], f32, tag="d_out")
            nc.sync.dma_start(out=d_in[:], in_=arin[:])
            nc.gpsimd.collective_compute(
                "AllReduce",
                OP.add,
                replica_groups=[list(range(n_cores))],
                ins=[d_in[:].opt()],
                outs=[d_out[:].opt()],
            )
            aro = statp.tile([C, 2], f32, tag="aro")
            nc.sync.dma_start(out=aro[:], in_=d_out[:])

            # ---------------- BN scale/shift (all [C,1], fp32)
            mean = statp.tile([C, 1], f32, tag="mean")
            nc.vector.tensor_scalar_mul(mean[:], aro[:, 0:1], 1.0 / ntot)
            ex2 = statp.tile([C, 1], f32, tag="ex2")
            nc.vector.tensor_scalar_mul(ex2[:], aro[:, 1:2], 1.0 / ntot)
            var = statp.tile([C, 1], f32, tag="var")
            nc.vector.tensor_mul(var[:], mean[:], mean[:])
            nc.vector.tensor_sub(var[:], ex2[:], var[:])
            veps = statp.tile([C, 1], f32, tag="veps")
            nc.vector.tensor_scalar_add(veps[:], var[:], BN_EPS)
            eps_t = statp.tile([C, 1], f32, tag="eps_t")
            nc.vector.memset(eps_t[:], BN_EPS)
            sd = statp.tile([C, 1], f32, tag="sd")
            nc.scalar.activation(out=sd[:], in_=var[:], func=AT.Sqrt, bias=eps_t[:])
            z = statp.tile([C, 1], f32, tag="z")
            nc.vector.reciprocal(z[:], sd[:])
            # one Newton step: z <- z * (1.5 - 0.5 * veps * z^2)
            nt = statp.tile([C, 1], f32, tag="nt")
            nc.vector.tensor_mul(nt[:], z[:], z[:])
            nc.vector.tensor_mul(nt[:], nt[:], veps[:])
            nc.vector.tensor_scalar(
                out=nt[:], in0=nt[:], scalar1=-0.5, scalar2=1.5,
                op0=OP.mult, op1=OP.add,
            )
            nc.vector.tensor_mul(z[:], z[:], nt[:])
            scale_t = statp.tile([C, 1], f32, tag="scale_t")
            nc.vector.tensor_mul(scale_t[:], gamma_s[:], z[:])
            shift_t = statp.tile([C, 1], f32, tag="shift_t")
            nc.vector.tensor_mul(shift_t[:], mean[:], scale_t[:])
            nc.vector.tensor_sub(shift_t[:], beta_s[:], shift_t[:])

            # ---------------- BN apply + ReLU + writeback
            for idx, (s, c, ot) in enumerate(out_tiles):
                yt = ypool.tile([C, TILE_F], bf16, tag="yt")
                if idx % 4 != 3:
                    nc.scalar.activation(
                        out=yt[:], in_=ot, func=AT.Relu,
                        scale=scale_t[:], bias=shift_t[:],
                    )
                else:
                    nc.vector.tensor_scalar(
                        out=yt[:], in0=ot,
                        scalar1=scale_t[:], scalar2=shift_t[:],
                        op0=OP.mult, op1=OP.add,
                    )
                    nc.vector.tensor_scalar_max(yt[:], yt[:], 0.0)
                nc.sync.dma_start(
                    out=y_d[s, :, c * TILE_F:(c + 1) * TILE_F], in_=yt[:],
                )

    nc.compile()
    return nc


_NC_CACHE = {}


def _get_nc(n_cores: int = N_CORES):
    if n_cores not in _NC_CACHE:
        _NC_CACHE[n_cores] = build_nc(n_cores)
    return _NC_CACHE[n_cores]


def make_in_maps(x: np.ndarray, gamma: np.ndarray, beta: np.ndarray,
                 n_cores: int = N_CORES):
    import ml_dtypes

    x_r = np.ascontiguousarray(
        np.asarray(x, dtype=np.float32).reshape(B, C, HW)
    ).astype(ml_dtypes.bfloat16)
    g = np.ascontiguousarray(np.asarray(gamma, dtype=np.float32).reshape(C, 1))
    b = np.ascontiguousarray(np.asarray(beta, dtype=np.float32).reshape(C, 1))
    ident = np.eye(C, dtype=ml_dtypes.bfloat16)
    crecip = _counts_recip()
    maps = []
    for core in range(n_cores):
        maps.append({
            "x": x_r[core * BL:(core + 1) * BL],
            "gamma": g,
            "beta": b,
            "ident": ident,
            "crecip": crecip,
        })
    return maps


def kernel(x, gamma, beta):
    from concourse import bass_utils

    nc = _get_nc(N_CORES)
    in_maps = make_in_maps(x, gamma, beta, N_CORES)
    res = bass_utils.run_bass_kernel_spmd(nc, in_maps, core_ids=list(range(N_CORES)))
    y = np.concatenate([res.results[c]["y"] for c in range(N_CORES)], axis=0)
    return y.reshape(B, C, H, W).astype(np.float32)


# revision 10
# speedup vs baseline: 1.3501x; 1.1097x over previous
"""DCM (dynamic conv module) Trainium2 kernel, bf16 multi-engine version v3.

Reference computation (per sample b, channel c):
  f[b,c,3,3]  = adaptive_avg_pool2d(x[b,c], 3)        # dynamic depthwise filter
  out[b,c]    = depthwise_conv3x3(x[b,c], f[b,c])     # zero padding 1
  y           = relu(batchnorm_train(out, gamma, beta))  # batch stats over (B,H,W)

Sharding: data-parallel over batch B=16 across 8 cores (2 samples/core).
Sync-BN via a [C,2] AllReduce of per-channel (sum, sumsq).

Engine assignment (measured costs; all bf16 except PSUM/stats, tol 2e-2):
  PE    : identity-fold of the DVE accumulator (start=True, full coverage)
          + the six dj=+-1 taps as 512-col diag matmuls (216 ns warm issue).
  DVE   : pooling reduce for plane 0, the three dj=0 taps (4B-aligned
          contiguous bf16 -> 2x mode; (0,0) via tensor_scalar at 4x),
          per-plane wrap-correction columns, ~half the BN apply (4x).
  ACT   : diag weights, PSUM->SBUF bf16 copy (+accum sum), square
          (+accum sumsq), other half of BN apply.
  GPSIMD: pooling for plane 1 via tensor_scalar accum pieces + f1 assembly
          (keeps the DVE free; gpsimd is otherwise idle).
Horizontal zero-padding: PE taps stream flat x so dj=+-1 taps wrap at row
ends; the wrap is removed by 6 per-plane column ops on the resident output
(batched over all chunks) instead of per-chunk fixups. Vertical padding is
exact via row-clipped matmul ranges.
"""

import numpy as np

# ---------------------------------------------------------------- constants
B, C, H, W = 16, 128, 128, 128
N_CORES = 8
BL = B // N_CORES          # samples per core
HW = H * W                 # 16384 free elems per plane
FS = 3
BN_EPS = 1e-5

ROWS = 16                  # output rows per psum tile
NCHUNK = H // ROWS         # 8 chunks per plane
TILE_F = ROWS * W          # 2048 free elems per psum tile
NPSUM = NCHUNK * BL        # psum tiles per core

# adaptive_avg_pool2d(3) bin boundaries (PyTorch convention)
SH = [(i * H) // FS for i in range(FS)]
EH = [-((-(i + 1) * H) // FS) for i in range(FS)]
SW = [(i * W) // FS for i in range(FS)]
EW = [-((-(i + 1) * W) // FS) for i in range(FS)]


def tidx(di, dj):
    return 3 * (di + 1) + (dj + 1)


# engine split: PE does all horizontal-shift taps, DVE the aligned dj=0 taps
PE_TAPS = [(-1, -1), (0, -1), (1, -1), (-1, 1), (0, 1), (1, 1)]
DVE_TAPS = [(0, 0), (-1, 0), (1, 0)]   # (0,0) first: full-range init

MM_N = 512                 # columns per matmul (one psum bank)
NSL = TILE_F // MM_N       # bank slices per psum tile

# plane-1 pooling pieces: (row-bin i, row_lo, row_hi) chunk-aligned splits
POOL_PIECES = []
for _c in range(NCHUNK):
    for _i in range(FS):
        lo = max(_c * ROWS, SH[_i])
        hi = min(_c * ROWS + ROWS, EH[_i])
        if lo < hi:
            POOL_PIECES.append((_i, lo, hi))

# BN-apply split: DVE (tensor_scalar 4x mode, 2 ops) vs ACT (1 relu op)
DVE_APPLY = {0, 2, 4, 6, 8, 10, 12, 14, 15}


def _counts_recip():
    cr = np.empty((C, FS * FS), dtype=np.float32)
    for i in range(FS):
        for j in range(FS):
            cr[:, 3 * i + j] = 1.0 / float((EH[i] - SH[i]) * (EW[j] - SW[j]))
    return cr


def build_nc(n_cores: int = N_CORES):
    """Build + compile the per-core Bass program (identical on all cores)."""
    import concourse.bacc as bacc
    import concourse.tile as tile
    from concourse import mybir

    f32 = mybir.dt.float32
    bf16 = mybir.dt.bfloat16
    AT = mybir.ActivationFunctionType
    OP = mybir.AluOpType
    AX = mybir.AxisListType

    ntot = float(n_cores * BL * HW)   # BN element count per channel

    nc = bacc.Bacc(
        "TRN2",
        target_bir_lowering=False,
        debug=False,
        num_devices=n_cores,
    )

    x_d = nc.dram_tensor("x", [BL, C, HW], bf16, kind="ExternalInput").ap()
    gamma_d = nc.dram_tensor("gamma", [C, 1], f32, kind="ExternalInput").ap()
    beta_d = nc.dram_tensor("beta", [C, 1], f32, kind="ExternalInput").ap()
    ident_d = nc.dram_tensor("ident", [C, C], bf16, kind="ExternalInput").ap()
    crecip_d = nc.dram_tensor("crecip", [C, FS * FS], f32, kind="ExternalInput").ap()
    y_d = nc.dram_tensor("y", [BL, C, HW], bf16, kind="ExternalOutput").ap()

    with tile.TileContext(nc) as tc, nc.allow_low_precision("bf16 by design"):
        with (
            tc.tile_pool(name="singles", bufs=1) as singles,
            tc.tile_pool(name="planes", bufs=1) as planes,
            tc.tile_pool(name="outres", bufs=1) as outres,
            tc.tile_pool(name="psum", bufs=2, space="PSUM") as psum,
            tc.tile_pool(name="accp", bufs=3) as accp,
            tc.tile_pool(name="ypool", bufs=4) as ypool,
            tc.tile_pool(name="colsp", bufs=1) as colsp,
            tc.tile_pool(name="fpool", bufs=2) as fpool,
            tc.tile_pool(name="diagp", bufs=2 * len(PE_TAPS)) as diagp,
            tc.tile_pool(name="statp", bufs=1) as statp,
            tc.tile_pool(name="dram", bufs=1, space="DRAM") as dram,
        ):
            # ---- plane-0 DMAs first (head-latency critical), then the small
            # constants (ident needed early for diag builds), then plane 1
            xs_t = [
                planes.tile([C, HW], bf16, tag=f"xs{s}", name=f"xs{s}")
                for s in range(BL)
            ]
            for c in range(NCHUNK):
                nc.sync.dma_start(
                    out=xs_t[0][:, c * TILE_F:(c + 1) * TILE_F],
                    in_=x_d[0, :, c * TILE_F:(c + 1) * TILE_F],
                )

            # ---- constants
            gamma_s = singles.tile([C, 1], f32, tag="gamma")
            nc.sync.dma_start(out=gamma_s[:], in_=gamma_d[:, :])
            beta_s = singles.tile([C, 1], f32, tag="beta")
            nc.sync.dma_start(out=beta_s[:], in_=beta_d[:, :])
            ident_s = singles.tile([C, C], bf16, tag="ident")
            nc.sync.dma_start(out=ident_s[:], in_=ident_d[:, :])
            crecip_s = singles.tile([C, FS * FS], f32, tag="crecip")
            nc.sync.dma_start(out=crecip_s[:], in_=crecip_d[:, :])

            for c in range(NCHUNK):
                nc.sync.dma_start(
                    out=xs_t[1][:, c * TILE_F:(c + 1) * TILE_F],
                    in_=x_d[1, :, c * TILE_F:(c + 1) * TILE_F],
                )

            sums = statp.tile([C, NPSUM], f32, tag="sums")
            sumsq = statp.tile([C, NPSUM], f32, tag="sumsq")

            # Two warm-up AllReduces at kernel start: the first absorbs the
            # one-time ncfw ramp/barrier, the second warms the steady path.
            warm = statp.tile([C, 2], f32, tag="warm")
            nc.gpsimd.memset(warm[:], 0.0)
            for w in range(2):
                dw_in = dram.tile([C, 2], f32, tag=f"dw_in{w}")
                dw_out = dram.tile([C, 2], f32, tag=f"dw_out{w}")
                nc.sync.dma_start(out=dw_in[:], in_=warm[:])
                nc.gpsimd.collective_compute(
                    "AllReduce",
                    OP.add,
                    replica_groups=[list(range(n_cores))],
                    ins=[dw_in[:].opt()],
                    outs=[dw_out[:].opt()],
                )

            # ---------------- pooling + filters
            fT_t, fneg_t, dgs_t = [], [], []

            # plane 0: DVE column-sum reduce (bf16 colS), f0 on DVE
            xs = xs_t[0]
            colS = colsp.tile([C, FS, H], bf16, tag="colS")
            for c in range(NCHUNK):
                xv = xs[:, c * TILE_F:(c + 1) * TILE_F].rearrange(
                    "p (r w) -> p r w", w=W
                )
                for j in range(FS):
                    nc.vector.tensor_reduce(
                        out=colS[:, j, c * ROWS:(c + 1) * ROWS],
                        in_=xv[:, :, SW[j]:EW[j]],
                        axis=AX.X,
                        op=OP.add,
                    )
            fT0 = fpool.tile([C, FS * FS], f32, tag="fT")
            for i in range(FS):
                for j in range(FS):
                    nc.vector.tensor_reduce(
                        out=fT0[:, 3 * i + j:3 * i + j + 1],
                        in_=colS[:, j, SH[i]:EH[i]],
                        axis=AX.X,
                        op=OP.add,
                    )
            nc.vector.tensor_mul(fT0[:], fT0[:], crecip_s[:])
            fneg0 = fpool.tile([C, FS * FS], f32, tag="fneg")
            nc.vector.tensor_scalar_mul(fneg0[:], fT0[:], -1.0)

            # plane 1: colS reduces are interleaved into the s0 conv loop
            # below (DVE cadence stays ahead of the PE), f1 assembled after.
            colS1 = colsp.tile([C, FS, H], bf16, tag="colS1")

            def diag_build(fT):
                dgs = []
                for (di, dj) in PE_TAPS:
                    t = tidx(di, dj)
                    dg = diagp.tile([C, C], bf16, tag="diag")
                    nc.scalar.activation(
                        out=dg[:], in_=ident_s[:], func=AT.Copy,
                        scale=fT[:, t:t + 1],
                    )
                    dgs.append(dg)
                return dgs

            fT_t = [fT0, None]
            fneg_t = [fneg0, None]
            dgs_t = [diag_build(fT0), None]

            # ---------------- conv chunks
            out_tiles = []
            for s in range(BL):
                if s == 1:
                    # assemble f1 from the interleaved colS1 reduces
                    fT1 = fpool.tile([C, FS * FS], f32, tag="fT")
                    for i in range(FS):
                        for j in range(FS):
                            nc.vector.tensor_reduce(
                                out=fT1[:, 3 * i + j:3 * i + j + 1],
                                in_=colS1[:, j, SH[i]:EH[i]],
                                axis=AX.X,
                                op=OP.add,
                            )
                    nc.vector.tensor_mul(fT1[:], fT1[:], crecip_s[:])
                    fneg1 = fpool.tile([C, FS * FS], f32, tag="fneg")
                    nc.vector.tensor_scalar_mul(fneg1[:], fT1[:], -1.0)
                    fT_t[1] = fT1
                    fneg_t[1] = fneg1
                    dgs_t[1] = diag_build(fT1)
                xs, fT, fneg, dgs = xs_t[s], fT_t[s], fneg_t[s], dgs_t[s]
                xr = xs[:].rearrange("p (r w) -> p r w", w=W)   # [C,H,W] view
                ot_full = outres.tile([C, HW], bf16, tag=f"ot{s}")
                for c in range(NCHUNK):
                    R0 = c * ROWS
                    kpt = s * NCHUNK + c

                    # ---- DVE dj=0 taps into bf16 accumulator (aligned)
                    acc = accp.tile([C, TILE_F], bf16, tag="acc")
                    nc.vector.tensor_scalar_mul(
                        acc[:], xs[:, R0 * W:R0 * W + TILE_F],
                        fT[:, tidx(0, 0):tidx(0, 0) + 1],
                    )
                    if s == 0:
                        # interleave plane-1 pooling on the DVE
                        xv1 = xs_t[1][:, c * TILE_F:(c + 1) * TILE_F].rearrange(
                            "p (r w) -> p r w", w=W
                        )
                        for j in range(FS):
                            nc.vector.tensor_reduce(
                                out=colS1[:, j, c * ROWS:(c + 1) * ROWS],
                                in_=xv1[:, :, SW[j]:EW[j]],
                                axis=AX.X,
                                op=OP.add,
                            )
                    for (di, dj) in DVE_TAPS[1:]:
                        t = tidx(di, dj)
                        rlo = max(R0, -di)
                        rhi = min(R0 + ROWS, H - di)
                        a0 = (rlo - R0) * W
                        a1 = (rhi - R0) * W
                        nc.vector.scalar_tensor_tensor(
                            out=acc[:, a0:a1],
                            in0=xs[:, (rlo + di) * W:(rhi + di) * W],
                            scalar=fT[:, t:t + 1],
                            in1=acc[:, a0:a1],
                            op0=OP.mult,
                            op1=OP.add,
                        )

                    # ---- PE: identity fold first (full coverage, start),
                    #      then the six dj=+-1 taps (row-clipped)
                    pt = psum.tile([C, TILE_F], f32, tag="pt")
                    for sl in range(NSL):
                        nc.tensor.matmul(
                            pt[:, sl * MM_N:(sl + 1) * MM_N],
                            ident_s[:],
                            acc[:, sl * MM_N:(sl + 1) * MM_N],
                            start=True,
                            stop=False,
                        )
                    for ti, (di, dj) in enumerate(PE_TAPS):
                        last = ti == len(PE_TAPS) - 1
                        for sl in range(NSL):
                            r0 = R0 + sl * 4
                            rlo = max(r0, -di)
                            rhi = min(r0 + 4, H - di)
                            if rlo >= rhi:
                                continue
                            oc0 = (rlo - R0) * W
                            oc1 = (rhi - R0) * W
                            m0 = (rlo + di) * W + dj
                            m1 = (rhi + di) * W + dj
                            if m0 < 0:
                                oc0 += 1
                                m0 = 0
                            if m1 > HW:
                                oc1 -= 1
                                m1 = HW
                            nc.tensor.matmul(
                                pt[:, oc0:oc1],
                                dgs[ti][:],
                                xs[:, m0:m1],
                                start=False,
                                stop=last and sl == NSL - 1,
                            )

                    # ---- PSUM -> resident SBUF bf16 copy, fused stats
                    ot = ot_full[:, c * TILE_F:(c + 1) * TILE_F]
                    nc.scalar.activation(
                        out=ot, in_=pt[:], func=AT.Copy,
                        accum_out=sums[:, kpt:kpt + 1],
                    )
                    nc.scalar.activation(
                        out=pt[:], in_=pt[:], func=AT.Square,
                        accum_out=sumsq[:, kpt:kpt + 1],
                    )
                    out_tiles.append((s, c, ot))

                # ---- per-plane wrap corrections on the resident output:
                # out(r, 0)   -= sum_di f[di,-1] * x[r+di-1, W-1]
                # out(r, W-1) -= sum_di f[di,+1] * x[r+di+1, 0]
                # (row ranges limited to where the matmuls added the wrap)
                otv = ot_full[:].rearrange("p (r w) -> p r w", w=W)
                for di in (-1, 0, 1):
                    rlo, rhi = max(0, 1 - di), min(H, H - di)
                    nc.vector.scalar_tensor_tensor(
                        out=otv[:, rlo:rhi, 0:1],
                        in0=xr[:, rlo + di - 1:rhi + di - 1, W - 1:W],
                        scalar=fneg[:, tidx(di, -1):tidx(di, -1) + 1],
                        in1=otv[:, rlo:rhi, 0:1],
                        op0=OP.mult,
                        op1=OP.add,
                    )
                    rlo, rhi = max(0, -di), min(H, H - 1 - di)
                    nc.vector.scalar_tensor_tensor(
                        out=otv[:, rlo:rhi, W - 1:W],
                        in0=xr[:, rlo + di + 1:rhi + di + 1, 0:1],
                        scalar=fneg[:, tidx(di, 1):tidx(di, 1) + 1],
                        in1=otv[:, rlo:rhi, W - 1:W],
                        op0=OP.mult,
                        op1=OP.add,
                    )

            # ---------------- sync-BN stats AllReduce
            arin = statp.tile([C, 2], f32, tag="arin")
            nc.vector.tensor_reduce(out=arin[:, 0:1], in_=sums[:], axis=AX.X, op=OP.add)
            nc.vector.tensor_reduce(out=arin[:, 1:2], in_=sumsq[:], axis=AX.X, op=OP.add)
            d_in = dram.tile([C, 2], f32, tag="d_in")
            d_out = dram.tile([C, 2], f32, tag="d_out")
            nc.sync.dma_start(out=d_in[:], in_=arin[:])
            nc.gpsimd.collective_compute(
                "AllReduce",
                OP.add,
                replica_groups=[list(range(n_cores))],
                ins=[d_in[:].opt()],
                outs=[d_out[:].opt()],
            )
            aro = statp.tile([C, 2], f32, tag="aro")
            nc.sync.dma_start(out=aro[:], in_=d_out[:])

            # ---------------- BN scale/shift (all [C,1], fp32)
            mean = statp.tile([C, 1], f32, tag="mean")
            nc.vector.tensor_scalar_mul(mean[:], aro[:, 0:1], 1.0 / ntot)
            ex2 = statp.tile([C, 1], f32, tag="ex2")
            nc.vector.tensor_scalar_mul(ex2[:], aro[:, 1:2], 1.0 / ntot)
            var = statp.tile([C, 1], f32, tag="var")
            nc.vector.tensor_mul(var[:], mean[:], mean[:])
            nc.vector.tensor_sub(var[:], ex2[:], var[:])
            veps = statp.tile([C, 1], f32, tag="veps")
            nc.vector.tensor_scalar_add(veps[:], var[:], BN_EPS)
            eps_t = statp.tile([C, 1], f32, tag="eps_t")
            nc.vector.memset(eps_t[:], BN_EPS)
            sd = statp.tile([C, 1], f32, tag="sd")
            nc.scalar.activation(out=sd[:], in_=var[:], func=AT.Sqrt, bias=eps_t[:])
            z = statp.tile([C, 1], f32, tag="z")
            nc.vector.reciprocal(z[:], sd[:])
            # one Newton step: z <- z * (1.5 - 0.5 * veps * z^2)
            nt = statp.tile([C, 1], f32, tag="nt")
            nc.vector.tensor_mul(nt[:], z[:], z[:])
            nc.vector.tensor_mul(nt[:], nt[:], veps[:])
            nc.vector.tensor_scalar(
                out=nt[:], in0=nt[:], scalar1=-0.5, scalar2=1.5,
                op0=OP.mult, op1=OP.add,
            )
            nc.vector.tensor_mul(z[:], z[:], nt[:])
            scale_t = statp.tile([C, 1], f32, tag="scale_t")
            nc.vector.tensor_mul(scale_t[:], gamma_s[:], z[:])
            shift_t = statp.tile([C, 1], f32, tag="shift_t")
            nc.vector.tensor_mul(shift_t[:], mean[:], scale_t[:])
            nc.vector.tensor_sub(shift_t[:], beta_s[:], shift_t[:])

            # ---------------- BN apply + ReLU + writeback (DVE 4x / ACT split)
            for idx, (s, c, ot) in enumerate(out_tiles):
                yt = ypool.tile([C, TILE_F], bf16, tag="yt")
                if idx in DVE_APPLY:
                    nc.vector.tensor_scalar(
                        out=yt[:], in0=ot,
                        scalar1=scale_t[:], scalar2=shift_t[:],
                        op0=OP.mult, op1=OP.add,
                    )
                    nc.vector.tensor_scalar_max(yt[:], yt[:], 0.0)
                else:
                    nc.scalar.activation(
                        out=yt[:], in_=ot, func=AT.Relu,
                        scale=scale_t[:], bias=shift_t[:],
                    )
                nc.sync.dma_start(
                    out=y_d[s, :, c * TILE_F:(c + 1) * TILE_F], in_=yt[:],
                )

    nc.compile()
    return nc


_NC_CACHE = {}


def _get_nc(n_cores: int = N_CORES):
    if n_cores not in _NC_CACHE:
        _NC_CACHE[n_cores] = build_nc(n_cores)
    return _NC_CACHE[n_cores]


def make_in_maps(x: np.ndarray, gamma: np.ndarray, beta: np.ndarray,
                 n_cores: int = N_CORES):
    import ml_dtypes

    x_r = np.ascontiguousarray(
        np.asarray(x, dtype=np.float32).reshape(B, C, HW)
    ).astype(ml_dtypes.bfloat16)
    g = np.ascontiguousarray(np.asarray(gamma, dtype=np.float32).reshape(C, 1))
    b = np.ascontiguousarray(np.asarray(beta, dtype=np.float32).reshape(C, 1))
    ident = np.eye(C, dtype=ml_dtypes.bfloat16)
    crecip = _counts_recip()
    maps = []
    for core in range(n_cores):
        maps.append({
            "x": x_r[core * BL:(core + 1) * BL],
            "gamma": g,
            "beta": b,
            "ident": ident,
            "crecip": crecip,
        })
    return maps


def kernel(x, gamma, beta):
    from concourse import bass_utils

    nc = _get_nc(N_CORES)
    in_maps = make_in_maps(x, gamma, beta, N_CORES)
    res = bass_utils.run_bass_kernel_spmd(nc, in_maps, core_ids=list(range(N_CORES)))
    y = np.concatenate([res.results[c]["y"] for c in range(N_CORES)], axis=0)
    return y.reshape(B, C, H, W).astype(np.float32)
